# revision 46
# baseline (speedup 1.0000x reference)
"""Windowed sparse attention (16x16 windows, keys from x+skip) on 8 TRN2 NeuronCores.

Reference computation (all 1x1 convs + per-window attention):
  q = Wq @ x;  k,v = split(Wkv @ [x;skip]);  per 16x16 window w/ 256 queries and
  512 keys (256 from x, 256 from skip):  out = softmax(q k^T / 8) v;  y = Wo @ out + bo.

Sharding: each core takes one 16-row strip of the 128x128 image (one window-row X),
both batch elements - all 128 of its windows are fully local; only weights replicated.

Per-core dataflow (PE-matmul-stream bound; measured HW costs drive the design):
  - q/k projections run in fp8e4 DoubleRow (weights scaled x16 on host to stay
    in e4m3 normal range; the 1/256 compensation is folded into the exp scale).
    One DoubleRow matmul contracts both 128-channel halves at 2 elem/cycle -
    2.0x measured over bf16.  PSUM evacuation alternates DVE/ScalarE so the
    copies keep up with the matmul stream (DVE alone was the q/k-phase limiter).
  - v is projected per window in bf16 (x chunk stationary; fp8 would cost ~5%
    output error, bf16 is exact enough) and woven into attention's exp gaps
  - simT[j,i] = kT^T @ qT per window: K=64 row-tiled pairs at tile_position
    (0,0)/(64,0) stream concurrently on HW (measured 64.8 ns/MM vs 221 serial)
  - exp on ScalarE (combined scale 0.125/256 folded in), written as bf16
  - attn@v via lhsT = [v | ones] bf16: softmax denominator arrives free as
    psum row 64; runs at pure stream rate (130 ns/MM measured)
  - recip(s) on DVE per window-pair, broadcast via DRAM-bounce DMA (f32),
    normalize-mult on GPSIMD (f32; DVE at the tail loses - it carries Wo evacs); Wo runs
    pair-batched (N=512, bf16) with the bias added in a DVE evac -
    ScalarE is exp-critical, so keeping Wo evacs out of its strict FIFO
    was worth ~33 us/body alone.

Measured on HW (interleaved A/B, slope timing): ~242 us/body vs ~296 us for
the all-bf16 DVE-evac ancestor. Timing noise on this setup is large (±8%
session drift); only interleaved comparisons are trustworthy.
"""
import sys

if '/opt/trn_rl_repo' not in sys.path:
    sys.path.insert(0, '/opt/trn_rl_repo')

import numpy as np
import ml_dtypes
import concourse.bass as bass
import concourse.tile as tile
import concourse.mybir as mybir
from concourse.bass_utils import run_bass_kernel_spmd

F32 = mybir.dt.float32
F32R = mybir.dt.float32r
BF16 = mybir.dt.bfloat16
FP8 = mybir.dt.float8e4
AFT = mybir.ActivationFunctionType
DR = mybir.MatmulPerfMode.DoubleRow

N_CORES = 8
B = 2            # batch
C = 256          # model channels
H = 8            # heads
D = 64           # head dim
INNER = H * D    # 512
WIN = 16         # window side
RS = 16          # strip rows per core (= one window row)
WCOL = 128       # image width
PX = RS * WCOL   # 2048 pixels per (batch, strip)
NY = 8           # windows along width
NI = WIN * WIN   # 256 queries per window
W8SCALE = 16.0   # fp8 weight pre-scale for q/k projections

# --- build-time config (ablation flags; module-level so harnesses can set) ---
QK_FP8 = True        # q/k projections via fp8 DoubleRow (else bf16 as v1)
QK_EVAC_DUAL = True  # alternate DVE/ScalarE for q/k psum evac (else DVE only)
PA_SEP = False       # av psum tiles in their own ring (else share "simpa")
PSIM_BUFS = 3        # bufs for the "simpa" psum ring (PSUM: 2*PSIM_BUFS+2*PA_SEP+2 <= 8)
BODIES_PER_ITER = 2  # bodies emitted per For_i iteration (reps<0 timing path)
STAGGERED = False    # staggered semaphore reset (no all-engine barrier per iter)
EVAC_MERGE = True    # one 1024-px evac per qk psum tile, engine alternating per tile
TAIL_DVE = False     # tail mults on GPSIMD: DVE now carries wo evacs at the tail (A/B: -22us)
BC_BUFS = 2          # bufs for the broadcast tile ring (2 overlaps tail chains)
WO_EVAC_DVE = True   # Wo bias evac on DVE (frees ScalarE, which is exp-critical)
QK_B1_DVE_ONLY = False  # tested: DVE-only for batch-1 qk evacs is ~30us WORSE (delays k_all)


def _split_multiwaits(nc, max_waits=1):
    """walrus codegen rejects instructions carrying >1 sem wait (seen on the
    TileContext exit drain); hoist extras onto single-wait NoOps just before."""
    for f in nc.m.functions:
        for blk in f.blocks:
            out, changed = [], False
            for ins in blk.instructions:
                si = ins.sync_info
                if si is not None and len(si.on_wait) > max_waits:
                    waits = list(si.on_wait)
                    SyncInfo = type(si)
                    extra, keep = waits[:-max_waits], waits[-max_waits:]
                    for i, w in enumerate(extra):
                        n = mybir.InstNoOp(name=f"{ins.name}-sw{i}", ins=[], outs=[])
                        n.engine = ins.engine
                        n.sync_info = SyncInfo(on_wait=[w], on_update=[])
                        out.append(n)
                    si.on_wait = keep
                    changed = True
                out.append(ins)
            if changed:
                blk.instructions = out
    return nc


class Ctx:
    pass


def _evac(nc, dst, src, ci, use_act=True):
    if ci == 0 or not QK_EVAC_DUAL or not use_act:
        nc.vector.tensor_copy(dst, src)
    else:
        nc.scalar.activation(dst, src, AFT.Identity, scale=1.0)


def _emit_qk_phase(g, b):
    """q/k projections for the whole batch-b strip (fp8 DoubleRow or bf16).
    q_all: [128(2h d), 4 hp, PX]; k_all: [128(2h d), 4 hp, 2 src, PX] (bf16).
    Evacuation alternates DVE (ci=0) / ScalarE (ci=1) per 512-px chunk."""
    if QK_FP8:
        return _emit_qk_phase_fp8(g, b)
    return _emit_qk_phase_bf16(g, b)


def _emit_qk_phase_fp8(g, b):
    nc = g.nc
    x8, s8 = g.strips8[b]
    srcs8 = (x8, s8)
    CW = 512
    # batch 1's phase is emitted mid-attention: ScalarE evacs there would sit
    # in the strict ACT FIFO ahead of window 8+'s exps, stalling the PE.
    use_act = (b == 0) or not QK_B1_DVE_ONLY

    q_all = g.qapool.tile([128, 4, PX], BF16, name=f"qa{b}", tag="qa")
    for mc in range(4):
        for hh in range(2):
            pq = g.psim.tile([128, 2, CW], F32, tag="simpa",
                             name=f"pq{b}_{mc}_{hh}")
            for ci in range(2):
                c0 = (hh * 2 + ci) * CW
                nc.tensor.matmul(
                    pq[:, ci], g.wq8_sb[:, :, mc * 128:(mc + 1) * 128],
                    x8[:, :, c0:c0 + CW], start=True, stop=True, perf_mode=DR)
            if EVAC_MERGE:
                _evac(nc, q_all[:, mc, hh * 2 * CW:(hh + 1) * 2 * CW], pq[:], hh, use_act)
            else:
                for ci in range(2):
                    c0 = (hh * 2 + ci) * CW
                    _evac(nc, q_all[:, mc, c0:c0 + CW], pq[:, ci], ci)

    k_all = g.kapool.tile([128, 4, 2, PX], BF16, name=f"ka{b}", tag="ka")
    for kc4 in range(4):
        for pi in range(2):
            for hh in range(2):
                pk = g.psim.tile([128, 2, CW], F32, tag="simpa",
                                 name=f"pk{b}_{kc4}_{pi}_{hh}")
                for ci in range(2):
                    c0 = (hh * 2 + ci) * CW
                    nc.tensor.matmul(
                        pk[:, ci], g.wk8_sb[:, :, kc4 * 128:(kc4 + 1) * 128],
                        srcs8[pi][:, :, c0:c0 + CW], start=True, stop=True,
                        perf_mode=DR)
                if EVAC_MERGE:
                    _evac(nc, k_all[:, kc4, pi, hh * 2 * CW:(hh + 1) * 2 * CW],
                          pk[:], hh, use_act)
                else:
                    for ci in range(2):
                        c0 = (hh * 2 + ci) * CW
                        _evac(nc, k_all[:, kc4, pi, c0:c0 + CW], pk[:, ci], ci)
    return q_all, k_all


def _emit_qk_phase_bf16(g, b):
    nc = g.nc
    x_sb, sk_sb = g.strips[b]
    srcs = (x_sb, sk_sb)
    NCH = 2              # px chunks of 1024
    CW = PX // NCH

    q_all = g.qapool.tile([128, 4, PX], BF16, name=f"qa{b}", tag="qa")
    for mc in range(4):
        for ch in range(NCH):
            pq = g.psim.tile([128, 2, CW // 2], F32, tag="simpa",
                             name=f"pq{b}_{mc}_{ch}")
            for h2 in range(2):       # psum bank halves (out <= 512 f32/bank)
                for kc in range(2):
                    c0 = ch * CW + h2 * (CW // 2)
                    nc.tensor.matmul(
                        pq[:, h2], g.wq_sb[:, kc, mc * 128:(mc + 1) * 128],
                        x_sb[:, kc, c0:c0 + CW // 2],
                        start=(kc == 0), stop=(kc == 1))
            if QK_EVAC_DUAL:
                for h2 in range(2):
                    c0 = ch * CW + h2 * (CW // 2)
                    _evac(nc, q_all[:, mc, c0:c0 + CW // 2], pq[:, h2], h2)
            else:
                nc.vector.tensor_copy(q_all[:, mc, ch * CW:(ch + 1) * CW], pq[:])

    k_all = g.kapool.tile([128, 4, 2, PX], BF16, name=f"ka{b}", tag="ka")
    for kc4 in range(4):
        for pi in range(2):
            for ch in range(NCH):
                pk = g.psim.tile([128, 2, CW // 2], F32, tag="simpa",
                                 name=f"pk{b}_{kc4}_{pi}_{ch}")
                for h2 in range(2):
                    for kc in range(2):
                        c0 = ch * CW + h2 * (CW // 2)
                        nc.tensor.matmul(
                            pk[:, h2], g.wk_sb[:, kc, kc4 * 128:(kc4 + 1) * 128],
                            srcs[pi][:, kc, c0:c0 + CW // 2],
                            start=(kc == 0), stop=(kc == 1))
                if QK_EVAC_DUAL:
                    for h2 in range(2):
                        c0 = ch * CW + h2 * (CW // 2)
                        _evac(nc, k_all[:, kc4, pi, c0:c0 + CW // 2], pk[:, h2], h2)
                else:
                    nc.vector.tensor_copy(k_all[:, kc4, pi, ch * CW:(ch + 1) * CW],
                                          pk[:])
    return q_all, k_all


def _emit_v(g, b, y):
    """v for window (b, y): [128 j, H, D+1] bf16 x 4 chunks (generator so the
    caller can weave the PE ops into attention's exp-latency gaps)."""
    nc = g.nc
    xwins = [g.strips[b][pi][:, :, y * NI:(y + 1) * NI] for pi in range(2)]
    v_y = []
    for half in range(2):
        for pi_jc in (half * 2, half * 2 + 1):
            pi, jc = pi_jc // 2, pi_jc % 2
            pv = g.pproj.tile([128, INNER], F32, tag="pp")
            for kc in range(2):
                nc.tensor.matmul(
                    pv[:], xwins[pi][:, kc, jc * 128:(jc + 1) * 128],
                    g.wv_sb[:, kc, :], start=(kc == 0), stop=(kc == 1))
            vt = g.vpool.tile([128, H, D + 1], BF16,
                              name=f"v{b}_{y}_{pi}{jc}", tag="v")
            nc.vector.tensor_copy(vt[:, :, D], nc.const_aps.tensor(1.0, (128, H), BF16))
            nc.vector.tensor_copy(
                vt[:, :, 0:D], pv[:].rearrange("p (h d) -> p h d", h=H))
            v_y.append(vt)
        yield None
    yield v_y


def _emit_attention(g, b, y, qk, v_y, filler=None):
    """simT -> exp -> (attn@[v|1]) for all 8 heads; returns unnorm [65, H, NI].

    The av matmuls for head-pair hp are emitted after the sim matmuls of
    hp+1, so ScalarE's exp latency hides under PE work instead of stalling
    the in-order PE stream."""
    nc = g.nc
    q_all, k_all = qk
    un = g.unpool.tile([65, H, NI], F32, name=f"u{b}_{y}", tag="un")

    def emit_sim(hp):
        sims = [g.psim.tile([128, 4, NI], F32, tag="simpa",
                            name=f"sim{b}_{y}_{hp}_{h2}") for h2 in range(2)]
        for pi in range(2):
            for jh in range(2):
                for h2 in range(2):
                    j0 = y * NI + jh * 128
                    nc.tensor.matmul(
                        sims[h2][:, pi * 2 + jh],
                        k_all[h2 * 64:(h2 + 1) * 64, hp, pi, j0:j0 + 128],
                        q_all[h2 * 64:(h2 + 1) * 64, hp, y * NI:(y + 1) * NI],
                        start=True, stop=True, tile_position=(h2 * 64, 0))
        exs = []
        for h2 in range(2):
            ex = g.expool.tile([128, 2, 2, NI], BF16, tag="ex")
            nc.scalar.activation(ex[:], sims[h2][:], AFT.Exp,
                                 scale=0.125 / (W8SCALE * W8SCALE) if QK_FP8 else 0.125)
            exs.append(ex)
        return exs

    def emit_av(hp, exs):
        pa = g.psim.tile([65, 2, NI], F32, tag="pa" if PA_SEP else "simpa",
                         name=f"pa{b}_{y}_{hp}", bufs=2 if PA_SEP else None)
        for h2 in range(2):
            h = hp * 2 + h2
            for n_mm, (pi, jc) in enumerate([(0, 0), (0, 1), (1, 0), (1, 1)]):
                nc.tensor.matmul(
                    pa[:, h2], v_y[pi * 2 + jc][:, h, :], exs[h2][:, pi, jc],
                    start=(n_mm == 0), stop=(n_mm == 3))
        nc.vector.tensor_copy(un[:, hp * 2:hp * 2 + 2], pa[:])

    prev = None
    for hp in range(4):
        exs = emit_sim(hp)
        if prev is not None:
            emit_av(prev[0], prev[1])
        if filler is not None:
            filler()
        prev = (hp, exs)
    emit_av(prev[0], prev[1])
    if filler is not None:
        filler()
    return un


def _emit_norm_chain(g, ts, unnorm, on_dve=False):
    """Stage 1 for a finished pair: recip(s), broadcast, normalize-mult on
    GPSIMD (or DVE for the tail pairs, when DVE is otherwise idle).
    ts are flat window indices (b*NY + y); returns the pair att tile
    [128, 4 hp, 2 win, NI] ready for the pair-batched Wo."""
    nc = g.nc
    mult_eng = nc.vector if on_dve else nc.gpsimd
    s_pair = g.spool.tile([2 * H, NI], F32, name=f"sm{ts[0]}", tag="sm")
    for i, tt in enumerate(ts):
        nc.sync.dma_start(s_pair[i * H:(i + 1) * H, :], unnorm[tt][64:65])
    s_rec = g.spool.tile([2 * H, NI], F32, name=f"sr{ts[0]}", tag="sr")
    nc.vector.reciprocal(s_rec[:], s_pair[:])
    s_dram = g.dpool.tile([2 * H, NI], F32, name=f"sd{ts[0]}", tag="sd")
    nc.sync.dma_start(s_dram[:], s_rec[:])

    bc = g.bcpool.tile([64, 2, H, NI], F32, name=f"bc{ts[0]}", tag="bc")
    APcls = type(bc[:])
    row = s_dram[0]
    nc.sync.dma_start(
        bc[:], APcls(tensor=row.tensor, offset=row.offset,
                     ap=[[0, 64], [NI, 2 * H], [1, NI]]))

    att2 = g.atpool.tile([128, 4, 2, NI], BF16, name=f"at{ts[0]}", tag="at")
    for i, tt in enumerate(ts):
        un = unnorm[tt]
        odd = g.atpool.tile([64, 4, NI], BF16, name=f"od{tt}", tag="od", bufs=2)
        # even heads -> partitions 0..63; odd heads -> temp, DMA to 64..127
        mult_eng.tensor_tensor(att2[0:64, :, i, :], un[0:64, 0:H:2],
                               bc[:, i, 0:H:2], mybir.AluOpType.mult)
        mult_eng.tensor_tensor(odd[:], un[0:64, 1:H:2], bc[:, i, 1:H:2],
                               mybir.AluOpType.mult)
        nc.sync.dma_start(att2[64:128, :, i, :], odd[:])
    return att2


def _emit_wo(g, ts, att2):
    """Stage 2: pair-batched Wo (N=512) + bias + store."""
    nc = g.nc
    b, y0 = ts[0] // NY, ts[0] % NY
    for mc in range(2):
        po = g.pproj.tile([128, 2, NI], F32, tag="pp")
        for hp in range(4):
            nc.tensor.matmul(po[:], g.wo_sb[:, hp, mc * 128:(mc + 1) * 128],
                             att2[:, hp], start=(hp == 0), stop=(hp == 3))
        ot = g.outpool.tile([128, 2, NI], F32, name=f"ot{ts[0]}_{mc}", tag="ot")
        if WO_EVAC_DVE:
            nc.vector.tensor_scalar_add(ot[:], po[:], g.bo_sb[:, mc:mc + 1])
        else:
            nc.scalar.activation(ot[:], po[:], AFT.Identity,
                                 bias=g.bo_sb[:, mc:mc + 1], scale=1.0)
        dst = g.out_d[b].rearrange("(mc p) r w -> p mc (r w)", p=128)
        nc.sync.dma_start(dst[:, mc, y0 * NI:(y0 + 2) * NI], ot[:])


def build_program(reps=1, phases=3, timing_mode=False):
    nc = bass.Bass("TRN2", target_bir_lowering=False, debug=False, num_devices=N_CORES)
    g = Ctx()
    g.nc = nc

    # timing_mode: all I/O tensors are device-Internal (initialized on device),
    # so repeated timing runs move ~nothing over the host link.
    KIN = "Internal" if timing_mode else "ExternalInput"
    KOUT = "Internal" if timing_mode else "ExternalOutput"
    x_d = nc.dram_tensor("x", [B, C, RS, WCOL], BF16, kind=KIN).ap()
    s_d = nc.dram_tensor("skip", [B, C, RS, WCOL], BF16, kind=KIN).ap()
    x8_d = nc.dram_tensor("x8", [B, C, RS, WCOL], FP8, kind=KIN).ap()
    s8_d = nc.dram_tensor("skip8", [B, C, RS, WCOL], FP8, kind=KIN).ap()
    wq8_d = nc.dram_tensor("wq8", [C, INNER], FP8, kind=KIN).ap()
    wk8_d = nc.dram_tensor("wk8", [C, INNER], FP8, kind=KIN).ap()
    wq_d = nc.dram_tensor("wqT", [C, INNER], BF16, kind=KIN).ap()
    wk_d = nc.dram_tensor("wkT", [C, INNER], BF16, kind=KIN).ap()
    wv_d = nc.dram_tensor("wvT", [C, INNER], BF16, kind=KIN).ap()
    wo_d = nc.dram_tensor("woT", [INNER, C], BF16, kind=KIN).ap()
    bo_d = nc.dram_tensor("bo", [C], F32, kind=KIN).ap()
    g.out_d = nc.dram_tensor("out", [B, C, RS, WCOL], F32, kind=KOUT).ap()
    if timing_mode:
        dummy_d = nc.dram_tensor("tdummy", [128], F32, kind="ExternalOutput").ap()

    with tile.TileContext(nc) as tc:
        with (
            tc.tile_pool(name="wpool", bufs=1) as wpool,
            tc.tile_pool(name="xpool", bufs=2) as xpool,
            tc.tile_pool(name="x8pool", bufs=2) as x8pool,
            tc.tile_pool(name="qa", bufs=1) as qapool,
            tc.tile_pool(name="ka", bufs=1) as kapool,
            tc.tile_pool(name="vp", bufs=9) as vpool,
            tc.tile_pool(name="ex", bufs=3) as expool,
            tc.tile_pool(name="un", bufs=4) as unpool,
            tc.tile_pool(name="at", bufs=2) as atpool,
            tc.tile_pool(name="bc", bufs=BC_BUFS) as bcpool,
            tc.tile_pool(name="sp", bufs=2) as spool,
            tc.tile_pool(name="ou", bufs=2) as outpool,
            tc.tile_pool(name="pproj", bufs=2, space="PSUM") as pproj,
            tc.tile_pool(name="psim", bufs=PSIM_BUFS, space="PSUM") as psim,
            tc.tile_pool(name="dram", bufs=3, space="DRAM") as dpool,
        ):
            g.qapool, g.kapool, g.vpool, g.expool = qapool, kapool, vpool, expool
            g.unpool, g.atpool, g.bcpool, g.spool = unpool, atpool, bcpool, spool
            g.outpool, g.pproj, g.psim, g.dpool = outpool, pproj, psim, dpool

            if timing_mode:
                # fill every Internal input tensor with a small constant via
                # broadcast DMA so the loop computes on benign finite values
                cb = wpool.tile([128, 512], BF16, name="cinit_b")
                nc.gpsimd.memset(cb[:], 0.01)
                c8 = wpool.tile([128, 512], FP8, name="cinit_8")
                nc.vector.tensor_copy(c8[:], cb[:])
                cf = wpool.tile([128, 512], F32, name="cinit_f")
                nc.vector.tensor_copy(cf[:], cb[:])
                APcls = type(cb[:])

                def bcast(src, n, w=512):
                    a = src[:]
                    return APcls(tensor=a.tensor, offset=a.offset,
                                 ap=[list(a.ap[0]), [0, n], [1, w]])

                for dst, src in ((x_d, cb), (s_d, cb), (x8_d, c8), (s8_d, c8)):
                    for bb in range(B):
                        for kc in range(2):
                            nc.sync.dma_start(
                                dst[bb].rearrange("(kc p) r w -> p kc (r w)",
                                                  p=128)[:, kc],
                                bcast(src, PX // 512))
                for dst, src in ((wq8_d, c8), (wk8_d, c8), (wq_d, cb),
                                 (wk_d, cb), (wv_d, cb), (wo_d, cb)):
                    view = dst.rearrange("(kc p) m -> p kc m", p=128)
                    kcn, m = view.shape[1], view.shape[2]
                    for kc in range(kcn):
                        nc.sync.dma_start(view[:, kc], bcast(src, m // 256, w=256))
                nc.sync.dma_start(bo_d.rearrange("(mc p) -> p mc", p=128),
                                  cf[:, 0:2])
                nc.sync.dma_start(dummy_d.rearrange("(a p) -> p a", p=128),
                                  cf[:, 0:1])

            if QK_FP8:
                g.wq8_sb = wpool.tile([128, 2, INNER], FP8, name="wq8")
                nc.sync.dma_start(g.wq8_sb[:], wq8_d.rearrange("(kc p) m -> p kc m", p=128))
                g.wk8_sb = wpool.tile([128, 2, INNER], FP8, name="wk8")
                nc.sync.dma_start(g.wk8_sb[:], wk8_d.rearrange("(kc p) m -> p kc m", p=128))
            else:
                g.wq_sb = wpool.tile([128, 2, INNER], BF16, name="wq")
                nc.sync.dma_start(g.wq_sb[:], wq_d.rearrange("(kc p) m -> p kc m", p=128))
                g.wk_sb = wpool.tile([128, 2, INNER], BF16, name="wk")
                nc.sync.dma_start(g.wk_sb[:], wk_d.rearrange("(kc p) m -> p kc m", p=128))
            g.wv_sb = wpool.tile([128, 2, INNER], BF16, name="wv")
            nc.sync.dma_start(g.wv_sb[:], wv_d.rearrange("(kc p) m -> p kc m", p=128))
            g.wo_sb = wpool.tile([128, 4, C], BF16, name="wo")
            nc.sync.dma_start(g.wo_sb[:], wo_d.rearrange("(kc p) m -> p kc m", p=128))
            g.bo_sb = wpool.tile([128, 2], F32, name="bo")
            nc.sync.dma_start(g.bo_sb[:], bo_d.rearrange("(mc p) -> p mc", p=128))

            def _body():
                g.strips = [_load_strips(g, b, x_d, s_d, xpool, "xs", "ss", BF16)
                            for b in range(B)]
                if QK_FP8:
                    g.strips8 = [_load_strips(g, b, x8_d, s8_d, x8pool, "x8", "s8", FP8)
                                 for b in range(B)]
                _emit_all(g, phases)

            if reps == 1:
                _body()
            elif reps < 0:
                with tc.For_i(0, -reps, 1, staggered_reset=STAGGERED):
                    for _ in range(BODIES_PER_ITER):
                        _body()
            else:
                with tc.For_i(0, reps, 1, staggered_reset=STAGGERED):
                    _body()

    _split_multiwaits(nc)
    return nc


def _load_strips(g, b, x_d, s_d, pool, tag_x, tag_s, dt):
    nc = g.nc
    x_sb = pool.tile([128, 2, PX], dt, name=f"{tag_x}{b}", tag=tag_x)
    nc.sync.dma_start(x_sb[:],
                      x_d[b].rearrange("(kc p) r w -> p kc (r w)", p=128))
    sk_sb = pool.tile([128, 2, PX], dt, name=f"{tag_s}{b}", tag=tag_s)
    nc.sync.dma_start(sk_sb[:],
                      s_d[b].rearrange("(kc p) r w -> p kc (r w)", p=128))
    return (x_sb, sk_sb)


def _emit_all(g, phases=3):
    """One continuous pipeline over all B*NY windows: weight-stationary q/k
    phase per batch, per-window v as the exp-latency filler, paired norm/Wo."""
    NW = B * NY
    bw = lambda t: (t // NY, t % NY)

    unnorm = {}
    chain_q = []   # window-pairs awaiting stage-1 (normalize chain)
    wo_q = []      # (ts, att2) awaiting stage-2 (Wo)
    def drain(gen):
        out = None
        for out in gen:
            pass
        return out
    qk = [None] * B
    qk[0] = _emit_qk_phase(g, 0)
    cur = drain(_emit_v(g, 0, 0))
    nxt_gen = None
    for t in range(NW):
        b, y = bw(t)
        if qk[b] is None:
            qk[b] = _emit_qk_phase(g, b)
        if phases < 2:
            if t + 1 < NW:
                cur = drain(_emit_v(g, *bw(t + 1)))
            continue
        nxt_gen = _emit_v(g, *bw(t + 1)) if t + 1 < NW else None
        nxt_result = [None]
        def filler():
            if nxt_gen is not None:
                r = next(nxt_gen, None)
                if r is not None:
                    nxt_result[0] = r
        unnorm[t] = _emit_attention(g, b, y, qk[b], cur, filler=filler)
        if nxt_gen is not None:
            r = drain(nxt_gen)
            if r is not None:
                nxt_result[0] = r
            cur = nxt_result[0]
        if phases < 3:
            continue
        if t % 2 == 1:
            chain_q.append((t - 1, t))
            if len(chain_q) > 1:
                ts = chain_q.pop(0)
                wo_q.append((ts, _emit_norm_chain(g, ts, unnorm)))
            if len(wo_q) > 1:
                ts, att2 = wo_q.pop(0)
                _emit_wo(g, ts, att2)
    for ts in chain_q:
        wo_q.append((ts, _emit_norm_chain(g, ts, unnorm, on_dve=TAIL_DVE)))
    for ts, att2 in wo_q:
        _emit_wo(g, ts, att2)


_PROGRAM = None


def _get_program():
    global _PROGRAM
    if _PROGRAM is None:
        _PROGRAM = build_program()
    return _PROGRAM


def _prep_weights(Wq, Wkv, Wo, bo):
    Wq = np.asarray(Wq, np.float32)
    Wkv = np.asarray(Wkv, np.float32)
    wqT = np.ascontiguousarray(Wq.T)
    wkT = np.ascontiguousarray(Wkv[:INNER].T)
    wq8 = np.clip(wqT * W8SCALE, -240, 240).astype(ml_dtypes.float8_e4m3)
    wk8 = np.clip(wkT * W8SCALE, -240, 240).astype(ml_dtypes.float8_e4m3)
    wvT = np.ascontiguousarray(Wkv[INNER:].T).astype(ml_dtypes.bfloat16)
    woT = np.ascontiguousarray(np.asarray(Wo, np.float32).T).astype(ml_dtypes.bfloat16)
    bo = np.ascontiguousarray(np.asarray(bo, np.float32))
    return wqT.astype(ml_dtypes.bfloat16), wkT.astype(ml_dtypes.bfloat16), \
        wq8, wk8, wvT, woT, bo


def make_in_maps(x, skip, Wq, Wkv, Wo, bo):
    x = np.asarray(x, dtype=np.float32)
    skip = np.asarray(skip, dtype=np.float32)
    wqT, wkT, wq8, wk8, wvT, woT, bo = _prep_weights(Wq, Wkv, Wo, bo)
    maps = []
    for c in range(N_CORES):
        r0, r1 = c * RS, (c + 1) * RS
        sx = _to_window_major(x[:, :, r0:r1, :])
        ss = _to_window_major(skip[:, :, r0:r1, :])
        maps.append({
            "x": sx.astype(ml_dtypes.bfloat16),
            "skip": ss.astype(ml_dtypes.bfloat16),
            "x8": np.clip(sx, -240, 240).astype(ml_dtypes.float8_e4m3),
            "skip8": np.clip(ss, -240, 240).astype(ml_dtypes.float8_e4m3),
            "wqT": wqT, "wkT": wkT,
            "wq8": wq8, "wk8": wk8, "wvT": wvT, "woT": woT, "bo": bo,
        })
    return maps


def kernel(x, skip, Wq, Wkv, Wo, bo):
    nc = _get_program()
    in_maps = make_in_maps(x, skip, Wq, Wkv, Wo, bo)
    res = run_bass_kernel_spmd(nc, in_maps, list(range(N_CORES)))
    out = np.empty((B, C, N_CORES * RS, WCOL), dtype=np.float32)
    for c in range(N_CORES):
        out[:, :, c * RS:(c + 1) * RS, :] = _from_window_major(res.results[c]["out"])
    return out


def _to_window_major(strip):
    # [B, C, 16, 128] row-major pixels -> pixel axis reordered to (y, r, c)
    s = strip.reshape(B, C, RS, NY, WIN).transpose(0, 1, 3, 2, 4)
    return np.ascontiguousarray(s).reshape(B, C, RS, WCOL)


def _from_window_major(strip):
    # inverse of _to_window_major
    s = strip.reshape(B, C, NY, RS, WIN).transpose(0, 1, 3, 2, 4)
    return np.ascontiguousarray(s).reshape(B, C, RS, WCOL)
